# revision 19
# baseline (speedup 1.0000x reference)
"""Trainium2 Bass kernel for nn_AttLayer (sparse attention with per-token
Q.K dot-product scores, softmax over sequence, and value pooling).

  reference:
    q = tanh(x @ Wq); k = tanh(x @ Wk); v = tanh(x @ Wv)      # [B,S,F]
    scores = sum_f q*k                                        # [B,S]
    p = softmax(scores, axis=S)[..., None]                    # [B,S,1]
    w = einsum("bsf,bso->bfo", v, p)                          # [B,F,1]
    returns (w, p)

Sharding: data-parallel over batch B=16 across 8 NeuronCores (2 each),
weights replicated; no collectives. x is transposed per-shard on the host
so the kernel streams xT[d, s] tiles straight into the PE array (the
contraction dim must live on SBUF partitions) with zero on-chip transposes.

Matmuls run in fp16 (m10) at full PE rate with fp32 PSUM accumulation —
bf16 (m8) fails the accuracy gate and float32r hangs this hardware/compiler
combination; simulated end-to-end rel err with fp16 inputs is ~1e-2 worst
element. V is stored bf16 in SBUF (value pooling is linear, error ~0.4%),
and softmax runs in fp32 with a global max subtraction (scores reach ~85).
"""

import numpy as np

import bass_rust
import concourse.bass as bass
import concourse.mybir as mybir
import concourse.tile as tile_mod
from concourse.bass_utils import run_bass_kernel_spmd
from concourse.tile import TileContext

# ---------------------------------------------------------------------------
# Problem constants (hardcoded per the harness contract)
# ---------------------------------------------------------------------------
B, S, D, F = 16, 4096, 1024, 1024
NCORES = 8
NB = B // NCORES  # batches per core
P = 128
DO = D // P  # d-chunks
NJ = S // P  # s-blocks of 128
FH = F // 2  # f half (512 = max fp32 PSUM moving dim)

f32 = mybir.dt.float32
f16 = mybir.dt.float16
bf16 = mybir.dt.bfloat16

# ---------------------------------------------------------------------------
# Workaround: this container's walrus build rejects more than ONE sync wait
# per instruction ("Too many sync wait commands"). Tile attaches several.
# Spread extras onto single-wait InstNoOp carriers placed just before the
# instruction on the same engine (same-engine streams are in-order, and sem
# values are monotone within a Tile section, so hoisting waits is sound).
# ---------------------------------------------------------------------------
_MAX_WAITS = 1
_uid = [0]


def _nop_with_wait(engine, wait):
    _uid[0] += 1
    return bass_rust.InstNoOp(
        name=f"waitcarrier-{_uid[0]}",
        engine=engine,
        text_hint="wait_carrier",
        bass_nofuse=True,
        sync_info=mybir.SyncInfo(on_wait=[wait], on_update=[]),
    )


_DMA_OPS = {"InstDMACopy", "InstTensorLoad", "InstTensorSave", "InstTrigger"}


def split_multi_waits(nc, aux=None):
    """Rewrite >1-wait instructions for this walrus's 1-wait limit.

    Compute-engine instructions block their sequencer on waits anyway, so
    extra waits hoist onto adjacent same-engine nop carriers (identical
    semantics). DMA instructions execute waits at the DGE queue level —
    hoisting onto the issuing engine's stream can deadlock it — so their
    waits are proxied through the otherwise-idle Pool engine: Pool nops
    absorb each wait in schedule order then bump the pre-allocated `aux`
    semaphore; the DMA waits only on aux reaching its chain index. Pool
    imposes order only on itself and nothing but these DMAs consumes aux,
    so no cycle is introduced.
    """
    n_split = 0
    aux_count = 0
    pool_eng = mybir.EngineType.Pool
    last_bb = None
    for f in nc.m.functions:
        for bb in f.blocks:
            insts = bb.instructions
            if insts:
                last_bb = bb
            out = []
            for inst in insts:
                si = inst.sync_info
                waits = list(si.on_wait) if si and si.on_wait else []
                if len(waits) > _MAX_WAITS:
                    n_split += 1
                    updates = list(si.on_update) if si.on_update else []
                    if type(inst).__name__ in _DMA_OPS:
                        assert aux is not None, "aux semaphore required"
                        aux_count += 1
                        for i, w in enumerate(waits):
                            _uid[0] += 1
                            upd = (
                                [
                                    mybir.SyncUpdate(
                                        sync_type="semaphore",
                                        id=aux.num,
                                        ant_name=aux.name,
                                        update_mode="sem-inc",
                                        update_value=1,
                                        update_reg=None,
                                    )
                                ]
                                if i == len(waits) - 1
                                else []
                            )
                            out.append(
                                bass_rust.InstNoOp(
                                    name=f"waitproxy-{_uid[0]}",
                                    engine=pool_eng,
                                    text_hint="dma_wait_proxy",
                                    bass_nofuse=True,
                                    sync_info=mybir.SyncInfo(
                                        on_wait=[w], on_update=upd
                                    ),
                                )
                            )
                        inst.sync_info = mybir.SyncInfo(
                            on_wait=[
                                mybir.SyncWait(
                                    sync_type="semaphore",
                                    id=aux.num,
                                    ant_name=aux.name,
                                    wait_mode="sem-ge-imm",
                                    wait_value=aux_count,
                                    wait_reg=None,
                                )
                            ],
                            on_update=updates,
                        )
                    else:
                        for w in waits[:-_MAX_WAITS]:
                            out.append(_nop_with_wait(inst.engine, w))
                        inst.sync_info = mybir.SyncInfo(
                            on_wait=waits[-_MAX_WAITS:],
                            on_update=updates,
                        )
                out.append(inst)
            if len(out) != len(insts):
                insts[:] = out
    if aux_count and last_bb is not None:
        _uid[0] += 1
        last_bb.instructions.append(
            bass_rust.InstNoOp(
                name=f"waitproxy-reset-{_uid[0]}",
                engine=pool_eng,
                text_hint="dma_wait_proxy_reset",
                bass_nofuse=True,
                sync_info=mybir.SyncInfo(
                    on_wait=[],
                    on_update=[
                        mybir.SyncUpdate(
                            sync_type="semaphore",
                            id=aux.num,
                            ant_name=aux.name,
                            update_mode="sem-sub-imm",
                            update_value=aux_count,
                            update_reg=None,
                        )
                    ],
                ),
            )
        )
    return n_split


def _patched_drain_and_barrier(self, tick_clock, wait_clock):
    nc = self.nc
    carrier = nc.sync.nop(nofuse=True, hint="drain_wait_carrier")
    wait_clock.add_sem_waits(
        carrier.ins, tile_mod.ScopedClock({None: tick_clock.global_clock})
    )
    waits = list(carrier.ins.sync_info.on_wait or [])
    if len(waits) > _MAX_WAITS:
        carrier.ins.sync_info = mybir.SyncInfo(on_wait=waits[:_MAX_WAITS], on_update=[])
        rest = waits[_MAX_WAITS:]
        for i in range(0, len(rest), _MAX_WAITS):
            extra = nc.sync.nop(nofuse=True, hint=f"drain_wait_carrier_{i}")
            extra.ins.sync_info = mybir.SyncInfo(
                on_wait=rest[i : i + _MAX_WAITS], on_update=[]
            )
    nc.sync.drain()

    nc.all_engine_barrier()
    assert self.sems is not None
    popped = nc._tile_sem_poison_stack.pop()
    assert popped is self._sem_poison
    nc.clear_and_free_semaphores(list(self.sems.allocated().values()))
    nc.all_engine_barrier()


tile_mod.TileContext._drain_and_barrier = _patched_drain_and_barrier


# ---------------------------------------------------------------------------
# Kernel graph
# ---------------------------------------------------------------------------
def build_nc():
    nc = bass.Bass("TRN2", target_bir_lowering=False, debug=False, num_devices=NCORES)
    # reserved before TileContext so its id never collides with Tile's sems
    aux_sem = nc.semaphore("dma_wait_proxy").__enter__()

    xt_ext = nc.declare_dram_parameter("xt", [NB, D, S], f16, isOutput=False)
    w_ext = {
        "q": nc.declare_dram_parameter("wq", [D, F], f16, isOutput=False),
        "k": nc.declare_dram_parameter("wk", [D, F], f16, isOutput=False),
        "v": nc.declare_dram_parameter("wv", [D, F], f16, isOutput=False),
    }
    id_ext = nc.declare_dram_parameter("ident", [P, P], f32, isOutput=False)
    outw_ext = nc.declare_dram_parameter("out_w", [NB, F], f32, isOutput=True)
    outp_ext = nc.declare_dram_parameter("out_p", [NB, S], f32, isOutput=True)

    AF = mybir.ActivationFunctionType
    AX = mybir.AxisListType
    OP = mybir.AluOpType

    with TileContext(nc) as tc:
        with (
            tc.tile_pool(name="consts", bufs=1) as consts,
            tc.tile_pool(name="w", bufs=1) as wpool,
            tc.tile_pool(name="v", bufs=1) as vpool,
            tc.tile_pool(name="x", bufs=2) as xpool,
            tc.tile_pool(name="qk", bufs=2) as qkpool,
            tc.tile_pool(name="s", bufs=2) as spool,
            tc.tile_pool(name="mm", bufs=1, space="PSUM") as mm,
            tc.tile_pool(name="tailps", bufs=1, space="PSUM") as tailps,
        ):
            ident = consts.tile([P, P], f32, tag="ident")
            nc.sync.dma_start(ident[:], id_ext[:])
            ones_row = consts.tile([1, P], f32, tag="ones_row")
            nc.vector.memset(ones_row[:], 1.0)
            ones_col = consts.tile([P, 1], f32, tag="ones_col")
            nc.vector.memset(ones_col[:], 1.0)
            # selector for the col-group-packed V^T@p reduction: 1.0 at
            # partitions 0/32/64/96 (the four col-group output rows)
            sel = consts.tile([P, 1], f32, tag="sel")
            nc.vector.memset(sel[:], 0.0)
            for jj in range(4):
                nc.vector.memset(sel[32 * jj : 32 * jj + 1, :], 1.0)

            def load_x(b, j):
                t = xpool.tile([P, DO, P], f16, tag="xt", name=f"xt_{b}_{j}")
                xt_b = xt_ext[b].rearrange("(do di) s -> di do s", di=P)
                nc.sync.dma_start(t[:], xt_b[:, :, j * P : (j + 1) * P])
                return t

            # Prefetch the first s-block before the 6 MB weight load so the
            # first matmul group's deps (x tile + all wq chunks) land early;
            # weights are emitted name-major because the j-loop consumes all
            # of wq (do inner) before touching wk/wv.
            x_pending = load_x(0, 0)
            w_sb = {}
            w_src = {}
            for name in ("q", "k", "v"):
                w_sb[name] = wpool.tile(
                    [P, DO, F], f16, tag=f"w_{name}", name=f"w_{name}"
                )
                w_src[name] = w_ext[name][:].rearrange("(do di) f -> di do f", di=P)
            # do-major emission: the j-loop is do-outer, so the first 6-matmul
            # burst needs only the three do=0 chunks (~0.8 MB), not 6 MB.
            for do in range(DO):
                for name in ("q", "k", "v"):
                    nc.sync.dma_start(w_sb[name][:, do, :], w_src[name][:, do, :])

            for b in range(NB):
                v_all = vpool.tile([P, NJ, F], bf16, tag="v_all")
                scores = spool.tile([P, NJ], f32, tag="scores")

                for j in range(NJ):
                    x_t = x_pending
                    if j + 1 < NJ:
                        x_pending = load_x(b, j + 1)
                    elif b + 1 < NB:
                        x_pending = load_x(b + 1, 0)
                    q_sb = qkpool.tile([P, F], f32, tag="q")
                    k_sb = qkpool.tile([P, F], f32, tag="k")
                    dests = {
                        "q": (q_sb, None),
                        "k": (k_sb, None),
                        "v": (None, v_all),
                    }
                    for gi, name in enumerate(("q", "k", "v")):
                        for h in range(2):
                            ps = mm.tile([P, FH], f32, tag=f"ps{gi}{h}")
                            for do in range(DO):
                                nc.tensor.matmul(
                                    ps[:],
                                    x_t[:, do, :],
                                    w_sb[name][:, do, h * FH : (h + 1) * FH],
                                    start=(do == 0),
                                    stop=(do == DO - 1),
                                )
                            dest, vdest = dests[name]
                            if dest is not None:
                                out_ap = dest[:, h * FH : (h + 1) * FH]
                            else:
                                out_ap = vdest[:, j, h * FH : (h + 1) * FH]
                            nc.scalar.activation(out_ap, ps[:], AF.Tanh)
                    qk = spool.tile([P, F], f32, tag="qkscratch")
                    nc.vector.scalar_tensor_tensor(
                        out=qk[:],
                        in0=q_sb[:],
                        scalar=1.0,
                        in1=k_sb[:],
                        op0=OP.mult,
                        op1=OP.mult,
                        accum_out=scores[:, j : j + 1],
                    )

                # ---- softmax over all S=4096 (scores laid out [128, 32]) ----
                m1 = spool.tile([P, 1], f32, tag="m1")
                nc.vector.reduce_max(m1[:], scores[:], axis=AX.X)
                # cross-partition max: PE transpose -> [1,128] -> DVE max
                t_mT = mm.tile([1, P], f32, tag="ps00")
                nc.tensor.transpose(t_mT[:], m1[:], ident[:])
                mg = spool.tile([1, 1], f32, tag="mg")
                nc.vector.reduce_max(mg[:], t_mT[:], axis=AX.X, negate=True)
                # broadcast -max to all partitions (ones-matmul)
                t_mb = mm.tile([P, 1], f32, tag="ps10")
                nc.tensor.matmul(t_mb[:], ones_row[:], mg[:], start=True, stop=True)
                nmb = spool.tile([P, 1], f32, tag="nmb")
                nc.scalar.copy(nmb[:], t_mb[:])
                # exp(scores - m) with fused per-partition row sums
                pu = spool.tile([P, NJ], f32, tag="pu")
                rowsum = spool.tile([P, 1], f32, tag="rowsum")
                nc.scalar.activation(
                    pu[:], scores[:], AF.Exp, bias=nmb[:], scale=1.0,
                    accum_out=rowsum[:],
                )
                # Z = sum over partitions (ones-matmul), then 1/Z broadcast
                t_z = mm.tile([1, 1], f32, tag="ps20")
                nc.tensor.matmul(t_z[:], rowsum[:], ones_col[:], start=True, stop=True)
                rz = spool.tile([1, 1], f32, tag="rz")
                nc.vector.reciprocal(rz[:], t_z[:])
                t_rzb = mm.tile([P, 1], f32, tag="ps11")
                nc.tensor.matmul(t_rzb[:], ones_row[:], rz[:], start=True, stop=True)
                pn = spool.tile([P, NJ], f32, tag="pn")
                nc.vector.tensor_scalar_mul(pn[:], pu[:], t_rzb[:, :1])

                # ---- attn_softmax output: transpose [128,32] -> [32,128] ----
                t_pT = mm.tile([NJ, P], f32, tag="ps01")
                nc.tensor.transpose(t_pT[:], pn[:], ident[:])
                p_out = spool.tile([NJ, P], f32, tag="p_out")
                nc.scalar.copy(p_out[:], t_pT[:])
                nc.sync.dma_start(
                    outp_ext[b].rearrange("(j f) -> j f", f=P), p_out[:]
                )

                # ---- weighted = V^T @ p (contraction over s on partitions) ----
                # The 32 per-block M=1 matmuls are packed 4-at-a-time into
                # distinct 32-column PE groups (tile_position), which stream
                # concurrently; the four partial rows (partitions 0/32/64/96)
                # are then folded with a selector ones-matmul.
                pb = spool.tile([P, NJ], bf16, tag="pb")
                nc.vector.tensor_copy(pb[:], pn[:])
                wout = spool.tile([1, F], f32, tag="wout")
                NSTEP = NJ // 4
                for h in range(2):
                    ps_w = tailps.tile(
                        [P, FH], f32, tag=f"wps{h}", name=f"wps{h}"
                    )
                    for step in range(NSTEP):
                        for jj in range(4):
                            j = step * 4 + jj
                            nc.tensor.matmul(
                                ps_w[32 * jj : 32 * jj + 1, :],
                                pb[:, j : j + 1],
                                v_all[:, j, h * FH : (h + 1) * FH],
                                start=(step == 0),
                                stop=(step == NSTEP - 1),
                                tile_position=(0, 32 * jj),
                            )
                    u_sb = spool.tile([P, FH], f32, tag="u", name=f"u{h}")
                    nc.scalar.copy(u_sb[:], ps_w[:])
                    ps_f = mm.tile([1, FH], f32, tag="ps20", name=f"psf{h}")
                    nc.tensor.matmul(ps_f[:], sel[:], u_sb[:], start=True, stop=True)
                    nc.scalar.copy(wout[:, h * FH : (h + 1) * FH], ps_f[:])
                nc.sync.dma_start(
                    outw_ext[b].rearrange("(a f) -> a f", a=1), wout[:]
                )

    split_multi_waits(nc, aux=aux_sem)
    return nc


_NC = None


def _get_nc():
    global _NC
    if _NC is None:
        _NC = build_nc()
    return _NC


def make_in_maps(x, wq, wk, wv):
    x = np.asarray(x, dtype=np.float32)
    wq = np.ascontiguousarray(np.asarray(wq, dtype=np.float32).astype(np.float16))
    wk = np.ascontiguousarray(np.asarray(wk, dtype=np.float32).astype(np.float16))
    wv = np.ascontiguousarray(np.asarray(wv, dtype=np.float32).astype(np.float16))
    ident = np.eye(P, dtype=np.float32)
    in_maps = []
    for c in range(NCORES):
        xb = x[c * NB : (c + 1) * NB]  # [NB, S, D]
        xt = np.ascontiguousarray(xb.transpose(0, 2, 1).astype(np.float16))
        in_maps.append(
            {"xt": xt, "wq": wq, "wk": wk, "wv": wv, "ident": ident}
        )
    return in_maps


def assemble(results):
    w = np.concatenate([results[c]["out_w"] for c in range(NCORES)], axis=0)
    p = np.concatenate([results[c]["out_p"] for c in range(NCORES)], axis=0)
    return w[..., None].astype(np.float32), p[..., None].astype(np.float32)


def kernel(x, W_key, W_query, W_value):
    nc = _get_nc()
    in_maps = make_in_maps(x, W_query, W_key, W_value)
    res = run_bass_kernel_spmd(nc, in_maps, core_ids=list(range(NCORES)))
    return assemble(res.results)
